# revision 14
# baseline (speedup 1.0000x reference)
"""Trainium2 Bass kernel for causal multi-head attention block.

Reference computation (fp32):
    qkv = x @ w_qkv;  q,k,v = split(qkv)
    attn = softmax(causal_mask(q k^T / sqrt(64)))
    out  = (attn @ v reassembled) @ w_out

Sharding over 8 NeuronCores: core c handles batch b = c//4 and heads
4*(c%4) .. 4*(c%4)+3 (4 of 16 heads).  Each core computes the rank-256
partial product of the output projection restricted to its heads'
channels; the host sums the 4 partials per batch.

v2: all-fp16 dataflow (inputs converted on host), block-interleaved
schedule: for each 512-row T block, project q/k/v, then run attention
for that query block (whose keys are now all available), with the
output projection lagged one block.  This keeps the PE dense across
phase boundaries (HAM stays warm) and hides the softmax exp (ACT) and
normalization chains under projection matmuls.
"""

import sys

for _p in ("/opt/trn_rl_repo", "/root/.axon_site/_ro/trn_rl_repo"):
    if _p not in sys.path:
        sys.path.append(_p)

import numpy as np

import concourse.bass as bass
import concourse.mybir as mybir
import concourse.tile as tile
from concourse import bacc, bass_utils

P = 128
B, T, C = 2, 2048, 1024
HPC = 4            # heads per core
DH = 64            # head dim
KT = C // P        # 8 contraction tiles over d_model
NQB = T // 512     # 4 query blocks of 512
NKT = T // P       # 16 key tiles of 128
F32 = mybir.dt.float32
F16 = mybir.dt.float16
EXP = mybir.ActivationFunctionType.Exp
LOG = mybir.ActivationFunctionType.Ln
SCALE = 1.0 / 8.0  # 1/sqrt(DH)


def _body(tc, nc, xT, wq, wk, wv, wo, tri, vones, out):
    with tc.tile_pool(name="const", bufs=1) as cpool:
        wq_sb = cpool.tile([P, KT, 2 * P], F16, name="wq_sb")
        wk_sb = cpool.tile([P, KT, 2 * P], F16, name="wk_sb")
        wv_sb = cpool.tile([P, KT, 2 * P], F16, name="wv_sb")
        wo_sb = cpool.tile([P, 2, C], F16, name="wo_sb")
        tri_sb = cpool.tile([P, P], F16, name="tri_sb")
        wqv = wq.rearrange("(kt p) n -> p kt n", p=P)
        wkv = wk.rearrange("(kt p) n -> p kt n", p=P)
        wvv = wv.rearrange("(kt p) n -> p kt n", p=P)
        xTv = xT.rearrange("(kt p) t -> p kt t", p=P)

        # persistent stores
        qT = [cpool.tile([P, T], F16, name=f"qT{pr}") for pr in range(2)]
        kT = [cpool.tile([P, T], F16, name=f"kT{pr}") for pr in range(2)]
        vS = cpool.tile([P, NKT, HPC, DH + 1], F16, name="vS")
        oT = [cpool.tile([P, T], F16, name=f"oT{pr}") for pr in range(2)]
        xts = [cpool.tile([P, KT, 512], F16, name=f"xt{i}") for i in range(NQB)]

        # ---- startup DMA: first q chain's inputs first, then the rest ----
        nc.sync.dma_start(wq_sb[:, 0:2], wqv[:, 0:2])
        nc.sync.dma_start(xts[0][:, 0:2, :], xTv[:, 0:2, 0:512])
        nc.sync.dma_start(wq_sb[:, 2:8], wqv[:, 2:8])
        nc.sync.dma_start(xts[0][:, 2:8, :], xTv[:, 2:8, 0:512])
        nc.sync.dma_start(wk_sb, wkv)
        nc.sync.dma_start(tri_sb, tri)
        nc.sync.dma_start(vS[:, :, :, DH : DH + 1], vones)
        nc.sync.dma_start(wv_sb, wvv)
        for later in range(1, NQB):
            nc.sync.dma_start(xts[later], xTv[:, :, later * 512 : (later + 1) * 512])
        nc.gpsimd.dma_start(wo_sb, wo.rearrange("(g p) n -> p g n", p=P))

        # preload the exp ACT table set during the startup DMA window
        warm = cpool.tile([1, 2], F32, name="warm")
        nc.vector.memset(warm, 1.0)
        nc.scalar.activation(warm, warm, EXP, scale=1.0)
        ones1 = cpool.tile([1, DH], F16, name="ones1")
        nc.vector.memset(ones1, 1.0)

        with (
            tc.tile_pool(name="qkp", bufs=2, space="PSUM") as qkp,
            tc.tile_pool(name="sps", bufs=2, space="PSUM") as sps,
            tc.tile_pool(name="ops", bufs=1, space="PSUM") as ops,
            tc.tile_pool(name="ptp", bufs=6) as ptp,
            tc.tile_pool(name="nrm", bufs=2) as nrm,
            tc.tile_pool(name="dsc", bufs=2, space="DRAM") as dsc,
            tc.tile_pool(name="osb", bufs=4) as osb,
        ):
            # PE warm-up: dummy matmuls on a zero tile during the startup DMA
            # window, so HAM releases the clock throttle before real work.
            wdum = cpool.tile([P, DH], F16, name="wdum")
            nc.vector.memset(wdum, 0.0)
            dum = qkp.tile([P, 512], F32, name="qk", tag="qk")
            for i in range(80):
                nc.tensor.matmul(
                    dum[0:DH, 0:DH], wdum, wdum, start=(i == 0), stop=(i == 79)
                )
            def emit_qkv(tb):
                """q/k/v projections for T block tb (both head pairs)."""
                xt = xts[tb]
                ts = slice(tb * 512, (tb + 1) * 512)
                for w_sb, dst in ((wq_sb, qT), (wk_sb, kT)):
                    for pr in range(2):
                        ps = qkp.tile([P, 512], F32, name="qk", tag="qk")
                        for kt in range(KT):
                            nc.tensor.matmul(
                                ps,
                                w_sb[:, kt, pr * P : (pr + 1) * P],
                                xt[:, kt, :],
                                start=(kt == 0),
                                stop=(kt == KT - 1),
                            )
                        nc.vector.tensor_copy(dst[pr][:, ts], ps)
                for sub in range(4):
                    tb1 = tb * 4 + sub
                    vp = qkp.tile([P, 512], F32, name="qk", tag="qk")
                    for kt in range(KT):
                        nc.tensor.matmul(
                            vp[:, 0 : 2 * P],
                            xt[:, kt, sub * P : (sub + 1) * P],
                            wv_sb[:, kt, :],
                            start=(kt == 0),
                            stop=(kt == KT - 1),
                        )
                    nc.vector.tensor_copy(
                        vS[:, tb1, :, 0:DH],
                        vp[:, 0 : 2 * P].rearrange("p (h d) -> p h d", d=DH),
                    )

            def emit_proj(tb):
                """output projection for T block tb (4 row tiles of 128)."""
                for tb1 in range(tb * 4, tb * 4 + 4):
                    for cb in range(2):
                        pp = qkp.tile([P, 512], F32, name="qk", tag="qk")
                        for pr in range(2):
                            nc.tensor.matmul(
                                pp,
                                oT[pr][:, tb1 * P : (tb1 + 1) * P],
                                wo_sb[:, pr, cb * 512 : (cb + 1) * 512],
                                start=(pr == 0),
                                stop=(pr == 1),
                            )
                        ot = osb.tile([P, 512], F16, name="ot")
                        nc.vector.tensor_copy(ot, pp)
                        nc.sync.dma_start(
                            out[tb1 * P : (tb1 + 1) * P, cb * 512 : (cb + 1) * 512],
                            ot,
                        )

            def emit_attn(qb, pr):
                """attention for query block qb, head pair pr."""
                op = ops.tile([DH + 1, 1024], F32, name="op", tag="op")
                nk = 4 * qb + 4

                def geom(j):
                    r = j - 4 * qb
                    width = 512 - r * P if r >= 0 else 512
                    col0 = r * P if r >= 0 else 0
                    return r, width, col0

                def emit_o(j, pts):
                    _, width, col0 = geom(j)
                    pt = pts.pop(j)
                    for h in range(2):
                        nc.tensor.matmul(
                            op[0 : DH + 1, h * 512 + col0 : (h + 1) * 512],
                            vS[:, j, pr * 2 + h, :],
                            pt[:, h * 512 : h * 512 + width],
                            start=(j == 0),
                            stop=(j == nk - 1),
                            skip_group_check=True,
                        )

                pts = {}
                for j in range(nk):
                    r, width, col0 = geom(j)
                    qoff = qb * 512 + col0
                    sp_ = sps.tile([P, 1024], F32, name="sp_", tag="sp")
                    for h in range(2):
                        nc.tensor.matmul(
                            sp_[:, h * 512 : h * 512 + width],
                            kT[pr][h * DH : (h + 1) * DH, j * P : (j + 1) * P],
                            qT[pr][h * DH : (h + 1) * DH, qoff : qoff + width],
                            start=True,
                            stop=True,
                        )
                    pt = ptp.tile([P, 1024], F16, name="pt")
                    s3 = sp_.rearrange("p (h w) -> p h w", h=2)[:, :, 0:width]
                    p3 = pt.rearrange("p (h w) -> p h w", h=2)[:, :, 0:width]
                    nc.scalar.activation(p3, s3, EXP, scale=SCALE)
                    if r >= 0:
                        for h in range(2):
                            nc.vector.tensor_mul(
                                pt[:, h * 512 : h * 512 + P],
                                pt[:, h * 512 : h * 512 + P],
                                tri_sb,
                            )
                    pts[j] = pt
                    if j > 1:
                        emit_o(j - 2, pts)
                if nk > 1:
                    emit_o(nk - 2, pts)
                emit_o(nk - 1, pts)

                # normalization: denominators ride in op row 64; 1/den is
                # computed on ACT as exp(-ln(den)) straight off the PSUM row
                # (no DRAM round trip), then partition-broadcast by DMA.
                qs = slice(qb * 512, (qb + 1) * 512)
                dTu = nrm.tile([1, 1024], F16, name="dTu", tag="dTu")
                nc.scalar.copy(dTu, op[DH : DH + 1, :])
                oTu = nrm.tile([DH, 2, 512], F16, name="oTu", tag="oTu")
                nc.vector.tensor_copy(
                    oTu, op.rearrange("p (h w) -> p h w", h=2)[0:DH]
                )
                # broadcast den across partitions with a K=1 ones matmul,
                # then fast-approx reciprocal straight out of PSUM.
                bc = nrm.tile([DH, 1024], F32, name="bc", tag="bc")
                for half in range(2):
                    hs = slice(half * 512, (half + 1) * 512)
                    bcp = qkp.tile([P, 512], F32, name="qk", tag="qk")
                    nc.tensor.matmul(
                        bcp[0:DH, :], ones1, dTu[:, hs], start=True, stop=True
                    )
                    nc.vector.reciprocal_approx_fast(bc[:, hs], bcp[0:DH, :])
                nc.vector.tensor_mul(oT[pr][0:DH, qs], oTu[0:DH, 0, :], bc[:, 0:512])
                o1 = nrm.tile([DH, 512], F16, name="o1", tag="o1")
                nc.vector.tensor_mul(o1, oTu[0:DH, 1, :], bc[:, 512:1024])
                nc.sync.dma_start(oT[pr][DH : 2 * DH, qs], o1)

            for tb in range(NQB):
                emit_qkv(tb)
                for pr in range(2):
                    emit_attn(tb, pr)
                if tb >= 1:
                    emit_proj(tb - 1)
            emit_proj(NQB - 1)


def build_bass():
    nc = bacc.Bacc("TRN2", target_bir_lowering=False, debug=False, num_devices=8)
    xT = nc.dram_tensor("xT", [C, T], F16, kind="ExternalInput").ap()
    wq = nc.dram_tensor("wq", [C, 2 * P], F16, kind="ExternalInput").ap()
    wk = nc.dram_tensor("wk", [C, 2 * P], F16, kind="ExternalInput").ap()
    wv = nc.dram_tensor("wv", [C, 2 * P], F16, kind="ExternalInput").ap()
    wo = nc.dram_tensor("wo", [2 * P, C], F16, kind="ExternalInput").ap()
    tri = nc.dram_tensor("tri", [P, P], F16, kind="ExternalInput").ap()
    vones = nc.dram_tensor(
        "vones", [P, NKT, HPC, 1], F16, kind="ExternalInput"
    ).ap()
    out = nc.dram_tensor("out", [T, C], F16, kind="ExternalOutput").ap()
    with tile.TileContext(nc) as tc:
        _body(tc, nc, xT, wq, wk, wv, wo, tri, vones, out)
    nc.compile()
    return nc


def make_in_maps(x, w_qkv, w_out):
    """Host-side sharding: returns the 8 per-core input dicts."""
    x = np.asarray(x, dtype=np.float32)
    w_qkv = np.asarray(w_qkv, dtype=np.float16)
    w_out = np.asarray(w_out, dtype=np.float16)
    kk = np.arange(P)
    tri = (kk[None, :] >= kk[:, None]).astype(np.float16)  # [k, q]: q >= k
    xTb = [np.ascontiguousarray(x[b].T.astype(np.float16)) for b in range(B)]
    in_maps = []
    for c in range(8):
        b = c // 4
        g = c % 4
        h0 = HPC * g * DH  # 256*g
        in_maps.append(
            {
                "xT": xTb[b],
                "wq": np.ascontiguousarray(w_qkv[:, h0 : h0 + 2 * P]),
                "wk": np.ascontiguousarray(w_qkv[:, C + h0 : C + h0 + 2 * P]),
                "wv": np.ascontiguousarray(w_qkv[:, 2 * C + h0 : 2 * C + h0 + 2 * P]),
                "wo": np.ascontiguousarray(w_out[h0 : h0 + 2 * P, :]),
                "tri": np.ascontiguousarray(tri),
                "vones": np.ones((P, NKT, HPC, 1), dtype=np.float16),
            }
        )
    return in_maps


_NC_CACHE = None
LAST_RESULTS = None  # BassKernelResults of the most recent run (for profiling)
TRACE = False


def kernel(x, w_qkv, w_out):
    global _NC_CACHE, LAST_RESULTS
    if _NC_CACHE is None:
        _NC_CACHE = build_bass()
    nc = _NC_CACHE
    in_maps = make_in_maps(x, w_qkv, w_out)
    res = bass_utils.run_bass_kernel_spmd(
        nc, in_maps, core_ids=list(range(8)), trace=TRACE
    )
    LAST_RESULTS = res
    out = np.zeros((B, T, C), dtype=np.float32)
    for c in range(8):
        out[c // 4] += res.results[c]["out"].astype(np.float32)
    return out


if __name__ == "__main__":
    # smoke test with random data
    rng = np.random.default_rng(0)
    x = rng.standard_normal((B, T, C), dtype=np.float32)
    w_qkv = rng.standard_normal((C, 3 * C), dtype=np.float32) / np.sqrt(C)
    w_out = rng.standard_normal((C, C), dtype=np.float32) / np.sqrt(C)
    o = kernel(x, w_qkv, w_out)
    print(o.shape, o.dtype)


# revision 16
# speedup vs baseline: 1.2034x; 1.2034x over previous
"""Trainium2 Bass kernel for causal multi-head attention block.

Reference computation (fp32):
    qkv = x @ w_qkv;  q,k,v = split(qkv)
    attn = softmax(causal_mask(q k^T / sqrt(64)))
    out  = (attn @ v reassembled) @ w_out

Sharding over 8 NeuronCores: core c handles batch b = c//4 and heads
4*(c%4) .. 4*(c%4)+3 (4 of 16 heads).  Each core computes the rank-256
partial product of the output projection restricted to its heads'
channels; the host sums the 4 partials per batch.

v2: all-fp16 dataflow (inputs converted on host), block-interleaved
schedule: for each 512-row T block, project q/k/v, then run attention
for that query block (whose keys are now all available), with the
output projection lagged one block.  This keeps the PE dense across
phase boundaries (HAM stays warm) and hides the softmax exp (ACT) and
normalization chains under projection matmuls.
"""

import sys

for _p in ("/opt/trn_rl_repo", "/root/.axon_site/_ro/trn_rl_repo"):
    if _p not in sys.path:
        sys.path.append(_p)

import numpy as np

import concourse.bass as bass
import concourse.mybir as mybir
import concourse.tile as tile
from concourse import bacc, bass_utils

P = 128
B, T, C = 2, 2048, 1024
HPC = 4            # heads per core
DH = 64            # head dim
KT = C // P        # 8 contraction tiles over d_model
NQB = T // 512     # 4 query blocks of 512
NKT = T // P       # 16 key tiles of 128
F32 = mybir.dt.float32
F16 = mybir.dt.float16
EXP = mybir.ActivationFunctionType.Exp
LOG = mybir.ActivationFunctionType.Ln
SCALE = 1.0 / 8.0  # 1/sqrt(DH)


def _body(tc, nc, xT, wq, wk, wv, wo, tri, vones, out):
    with tc.tile_pool(name="const", bufs=1) as cpool:
        wq_sb = cpool.tile([P, KT, 2 * P], F16, name="wq_sb")
        wk_sb = cpool.tile([P, KT, 2 * P], F16, name="wk_sb")
        wv_sb = cpool.tile([P, KT, 2 * P], F16, name="wv_sb")
        wo_sb = cpool.tile([P, 2, C], F16, name="wo_sb")
        tri_sb = cpool.tile([P, P], F16, name="tri_sb")
        wqv = wq.rearrange("(kt p) n -> p kt n", p=P)
        wkv = wk.rearrange("(kt p) n -> p kt n", p=P)
        wvv = wv.rearrange("(kt p) n -> p kt n", p=P)
        xTv = xT.rearrange("(kt p) t -> p kt t", p=P)

        # persistent stores
        qT = [cpool.tile([P, T], F16, name=f"qT{pr}") for pr in range(2)]
        kT = [cpool.tile([P, T], F16, name=f"kT{pr}") for pr in range(2)]
        vS = cpool.tile([P, NKT, HPC, DH + 1], F16, name="vS")
        oT = [cpool.tile([P, T], F16, name=f"oT{pr}") for pr in range(2)]
        xts = [cpool.tile([P, KT, 512], F16, name=f"xt{i}") for i in range(NQB)]

        # ---- startup DMA: first q chain's inputs first, then the rest ----
        nc.sync.dma_start(wq_sb[:, 0:2], wqv[:, 0:2])
        nc.sync.dma_start(xts[0][:, 0:2, :], xTv[:, 0:2, 0:512])
        nc.sync.dma_start(wq_sb[:, 2:8], wqv[:, 2:8])
        nc.sync.dma_start(xts[0][:, 2:8, :], xTv[:, 2:8, 0:512])
        nc.sync.dma_start(wk_sb, wkv)
        nc.sync.dma_start(tri_sb, tri)
        nc.sync.dma_start(vS[:, :, :, DH : DH + 1], vones)
        nc.sync.dma_start(wv_sb, wvv)
        for later in range(1, NQB):
            nc.sync.dma_start(xts[later], xTv[:, :, later * 512 : (later + 1) * 512])
        nc.gpsimd.dma_start(wo_sb, wo.rearrange("(g p) n -> p g n", p=P))

        # preload the exp ACT table set during the startup DMA window
        warm = cpool.tile([1, 2], F32, name="warm")
        nc.vector.memset(warm, 1.0)
        nc.scalar.activation(warm, warm, EXP, scale=1.0)
        ones1 = cpool.tile([1, DH], F16, name="ones1")
        nc.vector.memset(ones1, 1.0)

        with (
            tc.tile_pool(name="qkp", bufs=2, space="PSUM") as qkp,
            tc.tile_pool(name="sps", bufs=2, space="PSUM") as sps,
            tc.tile_pool(name="ops", bufs=1, space="PSUM") as ops,
            tc.tile_pool(name="ptp", bufs=6) as ptp,
            tc.tile_pool(name="nrm", bufs=2) as nrm,
            tc.tile_pool(name="dsc", bufs=2, space="DRAM") as dsc,
            tc.tile_pool(name="osb", bufs=4) as osb,
        ):
            # PE warm-up: dummy matmuls on a zero tile during the startup DMA
            # window, so HAM releases the clock throttle before real work.
            wdum = cpool.tile([P, DH], F16, name="wdum")
            nc.vector.memset(wdum, 0.0)
            dum = qkp.tile([P, 512], F32, name="qk", tag="qk")
            for i in range(80):
                nc.tensor.matmul(
                    dum[0:DH, 0:DH], wdum, wdum, start=(i == 0), stop=(i == 79)
                )
            def emit_qkv(tb):
                """q/k/v projections for T block tb (both head pairs)."""
                xt = xts[tb]
                ts = slice(tb * 512, (tb + 1) * 512)
                for w_sb, dst in ((wq_sb, qT), (wk_sb, kT)):
                    for pr in range(2):
                        ps = qkp.tile([P, 512], F32, name="qk", tag="qk")
                        for kt in range(KT):
                            nc.tensor.matmul(
                                ps,
                                w_sb[:, kt, pr * P : (pr + 1) * P],
                                xt[:, kt, :],
                                start=(kt == 0),
                                stop=(kt == KT - 1),
                            )
                        nc.vector.tensor_copy(dst[pr][:, ts], ps)
                for sub in range(4):
                    tb1 = tb * 4 + sub
                    vp = qkp.tile([P, 512], F32, name="qk", tag="qk")
                    for kt in range(KT):
                        nc.tensor.matmul(
                            vp[:, 0 : 2 * P],
                            xt[:, kt, sub * P : (sub + 1) * P],
                            wv_sb[:, kt, :],
                            start=(kt == 0),
                            stop=(kt == KT - 1),
                        )
                    nc.vector.tensor_copy(
                        vS[:, tb1, :, 0:DH],
                        vp[:, 0 : 2 * P].rearrange("p (h d) -> p h d", d=DH),
                    )

            def emit_proj(tb):
                """output projection for T block tb (4 row tiles of 128)."""
                for tb1 in range(tb * 4, tb * 4 + 4):
                    for cb in range(2):
                        pp = qkp.tile([P, 512], F32, name="qk", tag="qk")
                        for pr in range(2):
                            nc.tensor.matmul(
                                pp,
                                oT[pr][:, tb1 * P : (tb1 + 1) * P],
                                wo_sb[:, pr, cb * 512 : (cb + 1) * 512],
                                start=(pr == 0),
                                stop=(pr == 1),
                            )
                        ot = osb.tile([P, 512], F16, name="ot")
                        nc.vector.tensor_copy(ot, pp)
                        nc.sync.dma_start(
                            out[tb1 * P : (tb1 + 1) * P, cb * 512 : (cb + 1) * 512],
                            ot,
                        )

            def emit_attn(qb, pr):
                """attention for query block qb, head pair pr."""
                op = ops.tile([DH + 1, 1024], F32, name="op", tag="op")
                nk = 4 * qb + 4

                def geom(j):
                    r = j - 4 * qb
                    width = 512 - r * P if r >= 0 else 512
                    col0 = r * P if r >= 0 else 0
                    return r, width, col0

                def emit_o(j, pts):
                    _, width, col0 = geom(j)
                    pt = pts.pop(j)
                    for h in range(2):
                        nc.tensor.matmul(
                            op[0 : DH + 1, h * 512 + col0 : (h + 1) * 512],
                            vS[:, j, pr * 2 + h, :],
                            pt[:, h * 512 : h * 512 + width],
                            start=(j == 0),
                            stop=(j == nk - 1),
                            skip_group_check=True,
                        )

                pts = {}
                for j in range(nk):
                    r, width, col0 = geom(j)
                    qoff = qb * 512 + col0
                    sp_ = sps.tile([P, 1024], F32, name="sp_", tag="sp")
                    for h in range(2):
                        nc.tensor.matmul(
                            sp_[:, h * 512 : h * 512 + width],
                            kT[pr][h * DH : (h + 1) * DH, j * P : (j + 1) * P],
                            qT[pr][h * DH : (h + 1) * DH, qoff : qoff + width],
                            start=True,
                            stop=True,
                        )
                    pt = ptp.tile([P, 1024], F16, name="pt")
                    s3 = sp_.rearrange("p (h w) -> p h w", h=2)[:, :, 0:width]
                    p3 = pt.rearrange("p (h w) -> p h w", h=2)[:, :, 0:width]
                    nc.scalar.activation(p3, s3, EXP, scale=SCALE)
                    if r >= 0:
                        for h in range(2):
                            nc.vector.tensor_mul(
                                pt[:, h * 512 : h * 512 + P],
                                pt[:, h * 512 : h * 512 + P],
                                tri_sb,
                            )
                    pts[j] = pt
                    if j > 1:
                        emit_o(j - 2, pts)
                if nk > 1:
                    emit_o(nk - 2, pts)
                emit_o(nk - 1, pts)

                # normalization: denominators ride in op row 64; 1/den is
                # computed on ACT as exp(-ln(den)) straight off the PSUM row
                # (no DRAM round trip), then partition-broadcast by DMA.
                qs = slice(qb * 512, (qb + 1) * 512)
                dTu = nrm.tile([1, 1024], F16, name="dTu", tag="dTu")
                nc.scalar.copy(dTu, op[DH : DH + 1, :])
                oTu = nrm.tile([DH, 2, 512], F16, name="oTu", tag="oTu")
                nc.vector.tensor_copy(
                    oTu, op.rearrange("p (h w) -> p h w", h=2)[0:DH]
                )
                if qb == NQB - 1 and pr == 1:
                    # final normalization is on the critical path: broadcast
                    # den across partitions with a K=1 ones matmul, then
                    # fast-approx reciprocal straight out of PSUM (no DRAM
                    # hops).  Mid-kernel chains use the DMA round trip so
                    # the PE instruction stream never waits on them.
                    bc = nrm.tile([DH, 1024], F32, name="bc", tag="bc")
                    for half in range(2):
                        hs = slice(half * 512, (half + 1) * 512)
                        bcp = qkp.tile([P, 512], F32, name="qk", tag="qk")
                        nc.tensor.matmul(
                            bcp[0:DH, :], ones1, dTu[:, hs], start=True, stop=True
                        )
                        nc.vector.reciprocal_approx_fast(bc[:, hs], bcp[0:DH, :])
                else:
                    dd = dsc.tile([1024], F16, name="dd", tag="dd")
                    nc.sync.dma_start(dd[None], dTu)
                    rsh = nrm.tile([P, 8], F16, name="rsh", tag="rsh")
                    nc.sync.dma_start(rsh, dd.rearrange("(p c) -> p c", p=P))
                    rr = nrm.tile([P, 8], F16, name="rr", tag="rr")
                    with nc.allow_low_precision(reason="fp16 softmax denom"):
                        nc.vector.reciprocal(rr, rsh)
                    dd2 = dsc.tile([1024], F16, name="dd2", tag="dd2")
                    nc.sync.dma_start(dd2.rearrange("(p c) -> p c", p=P), rr)
                    bc = nrm.tile([DH, 1024], F16, name="bch", tag="bch")
                    nc.sync.dma_start(
                        bc[:, 0:512], dd2[None, 0:512].to_broadcast([DH, 512])
                    )
                    nc.sync.dma_start(
                        bc[:, 512:1024], dd2[None, 512:1024].to_broadcast([DH, 512])
                    )
                nc.vector.tensor_mul(oT[pr][0:DH, qs], oTu[0:DH, 0, :], bc[:, 0:512])
                o1 = nrm.tile([DH, 512], F16, name="o1", tag="o1")
                nc.vector.tensor_mul(o1, oTu[0:DH, 1, :], bc[:, 512:1024])
                nc.sync.dma_start(oT[pr][DH : 2 * DH, qs], o1)

            for tb in range(NQB):
                emit_qkv(tb)
                for pr in range(2):
                    emit_attn(tb, pr)
                if tb >= 1:
                    emit_proj(tb - 1)
            emit_proj(NQB - 1)


def build_bass():
    nc = bacc.Bacc("TRN2", target_bir_lowering=False, debug=False, num_devices=8)
    xT = nc.dram_tensor("xT", [C, T], F16, kind="ExternalInput").ap()
    wq = nc.dram_tensor("wq", [C, 2 * P], F16, kind="ExternalInput").ap()
    wk = nc.dram_tensor("wk", [C, 2 * P], F16, kind="ExternalInput").ap()
    wv = nc.dram_tensor("wv", [C, 2 * P], F16, kind="ExternalInput").ap()
    wo = nc.dram_tensor("wo", [2 * P, C], F16, kind="ExternalInput").ap()
    tri = nc.dram_tensor("tri", [P, P], F16, kind="ExternalInput").ap()
    vones = nc.dram_tensor(
        "vones", [P, NKT, HPC, 1], F16, kind="ExternalInput"
    ).ap()
    out = nc.dram_tensor("out", [T, C], F16, kind="ExternalOutput").ap()
    with tile.TileContext(nc) as tc:
        _body(tc, nc, xT, wq, wk, wv, wo, tri, vones, out)
    nc.compile()
    return nc


def make_in_maps(x, w_qkv, w_out):
    """Host-side sharding: returns the 8 per-core input dicts."""
    x = np.asarray(x, dtype=np.float32)
    w_qkv = np.asarray(w_qkv, dtype=np.float16)
    w_out = np.asarray(w_out, dtype=np.float16)
    kk = np.arange(P)
    tri = (kk[None, :] >= kk[:, None]).astype(np.float16)  # [k, q]: q >= k
    xTb = [np.ascontiguousarray(x[b].T.astype(np.float16)) for b in range(B)]
    in_maps = []
    for c in range(8):
        b = c // 4
        g = c % 4
        h0 = HPC * g * DH  # 256*g
        in_maps.append(
            {
                "xT": xTb[b],
                "wq": np.ascontiguousarray(w_qkv[:, h0 : h0 + 2 * P]),
                "wk": np.ascontiguousarray(w_qkv[:, C + h0 : C + h0 + 2 * P]),
                "wv": np.ascontiguousarray(w_qkv[:, 2 * C + h0 : 2 * C + h0 + 2 * P]),
                "wo": np.ascontiguousarray(w_out[h0 : h0 + 2 * P, :]),
                "tri": np.ascontiguousarray(tri),
                "vones": np.ones((P, NKT, HPC, 1), dtype=np.float16),
            }
        )
    return in_maps


_NC_CACHE = None
LAST_RESULTS = None  # BassKernelResults of the most recent run (for profiling)
TRACE = False


def kernel(x, w_qkv, w_out):
    global _NC_CACHE, LAST_RESULTS
    if _NC_CACHE is None:
        _NC_CACHE = build_bass()
    nc = _NC_CACHE
    in_maps = make_in_maps(x, w_qkv, w_out)
    res = bass_utils.run_bass_kernel_spmd(
        nc, in_maps, core_ids=list(range(8)), trace=TRACE
    )
    LAST_RESULTS = res
    out = np.zeros((B, T, C), dtype=np.float32)
    for c in range(8):
        out[c // 4] += res.results[c]["out"].astype(np.float32)
    return out


if __name__ == "__main__":
    # smoke test with random data
    rng = np.random.default_rng(0)
    x = rng.standard_normal((B, T, C), dtype=np.float32)
    w_qkv = rng.standard_normal((C, 3 * C), dtype=np.float32) / np.sqrt(C)
    w_out = rng.standard_normal((C, C), dtype=np.float32) / np.sqrt(C)
    o = kernel(x, w_qkv, w_out)
    print(o.shape, o.dtype)
